# revision 22
# baseline (speedup 1.0000x reference)
"""MoE logistic regression kernel for 8 Trainium2 NeuronCores.

Math (after dead-code elimination of the reference's unused router path):
    noise_logits = x @ noise_w.T + noise_b            # [B, E]
    top8 = top_k(noise_logits, 8)
    gates = softmax over the top-8 entries (others 0)
    expert = sigmoid(x @ expert_w.T + expert_b)       # [B, E]
    out[b] = sum_e gates[b,e] * expert[b,e]           # [B, 1]

Sharding: batch split 8 ways (2048 rows/core); weights replicated.

Key implementation choices:
- x is transposed on the host so each core streams contiguous [D, BC]
  chunks with D on partitions; no on-chip transpose of x.
- x and w are split into fp16 (hi, lo) pairs on the host (exact to ~22
  mantissa bits). The matmul runs 3 fp16 passes (hi@wh + lo@wh + hi@wl)
  accumulating in fp32 PSUM: ~fp32 accuracy at 3/4 the fp32 PE cost.
  (The top-8 selection margins require ~1e-6 logit accuracy: the
  smallest 8th/9th gap over the whole fixed batch is 8.8e-6.)
- noise_w/expert_w are concatenated into one 128-wide stationary operand
  so x streams through the PE once per (chunk, pass) for both matmuls;
  biases are added per-partition by the ACT epilogue ops.
- top-8 per row via the DVE Max8 + MatchReplace8 instructions; gates via
  exp(v - m1) with the (e_all - e_zap) trick which is exactly zero off
  the top-8; final dot + 1/Z normalization per 128-row tile.
"""

import sys

import numpy as np

if "/opt/trn_rl_repo" not in sys.path:
    sys.path.insert(0, "/opt/trn_rl_repo")

B, D, E, TOPK, NCORES = 16384, 4096, 64, 8, 8
BC = B // NCORES      # batch rows per core
BT = 512              # batch tile (one PSUM bank of fp32)
NT = BC // BT         # batch tiles per core
NK = D // 128         # contraction chunks
NEG_BIG = -1e30

_cached = {}


def _build_program(mm_dtype="fp16x2"):
    import concourse.bass as bass
    import concourse.tile as tile
    from concourse import bacc, mybir
    from concourse.masks import make_identity

    f32 = mybir.dt.float32
    f16 = mybir.dt.float16
    split = mm_dtype == "fp16x2"
    wdt = f16 if split else getattr(mybir.dt, mm_dtype)
    act = mybir.ActivationFunctionType

    nc = bacc.Bacc("TRN2", target_bir_lowering=False, debug=False)
    if split:
        # x as fp16 (hi, lo): [D, NT, 2, BT]; w pair pre-swizzled so the
        # SBUF image [128, NK*2*128] is one contiguous DMA.
        xt = nc.dram_tensor("xt", [D, NT, 2, BT], f16, kind="ExternalInput").ap()
        wt = nc.dram_tensor("wt", [128, NK * 2 * 128], f16,
                            kind="ExternalInput").ap()
    else:
        xt = nc.dram_tensor("xt", [D, NT, BT], f32, kind="ExternalInput").ap()
        wt = nc.dram_tensor("wt", [128, NK * 128], f32, kind="ExternalInput").ap()
    bb = nc.dram_tensor("bb", [128, 1], f32, kind="ExternalInput").ap()
    out = nc.dram_tensor("out", [BC, 1], f32, kind="ExternalOutput").ap()

    with tile.TileContext(nc) as tc:
        with (
            tc.tile_pool(name="consts", bufs=1) as consts,
            tc.tile_pool(name="xpool", bufs=6) as xpool,
            tc.tile_pool(name="eppool", bufs=2) as eppool,
            tc.tile_pool(name="small", bufs=3) as small,
            tc.tile_pool(name="psacc", bufs=1, space=bass.MemorySpace.PSUM) as psacc,
            tc.tile_pool(name="pstr", bufs=2, space=bass.MemorySpace.PSUM) as pstr,
            tc.tile_pool(name="psfin", bufs=1, space=bass.MemorySpace.PSUM) as psfin,
        ):
            # ---- constants ----
            if split:
                wt_sb = consts.tile([128, NK, 2, 128], wdt)
            else:
                wt_sb = consts.tile([128, NK, 128], wdt)
            nc.sync.dma_start(out=wt_sb, in_=wt)
            bb_sb = consts.tile([128, 1], f32)
            nc.sync.dma_start(out=bb_sb, in_=bb)
            ident = consts.tile([128, 128], f32)
            make_identity(nc, ident)
            final_sb = consts.tile([128, NT * 4], f32)

            # ---- matmuls: acc[t][0:64,:] = noise logits.T (pre-bias),
            #               acc[t][64:128,:] = expert logits.T (pre-bias)
            accs = [psacc.tile([128, BT], f32, tag=f"acc{t}", name=f"acc{t}")
                    for t in range(NT)]
            xview = xt.rearrange("(nk p) nt two b -> nk p nt two b", p=128) \
                if split else xt.rearrange("(nk p) nt b -> nk p nt b", p=128)
            for k in range(NK):
                if split:
                    xk = xpool.tile([128, NT, 2, BT], wdt, tag="xk")
                    nc.sync.dma_start(out=xk, in_=xview[k])
                    wh = wt_sb[:, k, 0, :]
                    wl = wt_sb[:, k, 1, :]
                    for t in range(NT):
                        nc.tensor.matmul(accs[t], lhsT=wh, rhs=xk[:, t, 0, :],
                                         start=(k == 0), stop=False)
                        nc.tensor.matmul(accs[t], lhsT=wh, rhs=xk[:, t, 1, :],
                                         start=False, stop=False)
                        nc.tensor.matmul(accs[t], lhsT=wl, rhs=xk[:, t, 0, :],
                                         start=False, stop=(k == NK - 1))
                else:
                    xk = xpool.tile([128, NT, BT], wdt, tag="xk")
                    nc.sync.dma_start(out=xk, in_=xview[k])
                    for t in range(NT):
                        nc.tensor.matmul(accs[t], lhsT=wt_sb[:, k, :],
                                         rhs=xk[:, t, :],
                                         start=(k == 0), stop=(k == NK - 1))

            # ---- epilogue per batch tile ----
            for t in range(NT):
                noiseT = eppool.tile([64, BT], f32, tag="noiseT")
                nc.scalar.add(noiseT, accs[t][0:64, :], bb_sb[0:64, :])
                eoT = eppool.tile([64, BT], f32, tag="eoT")
                nc.scalar.activation(eoT, accs[t][64:128, :],
                                     func=act.Sigmoid, bias=bb_sb[64:128, :])
                # transpose to batch-major: [128 batch, j | 4+j, 64]
                ps_ne = pstr.tile([128, 8, 64], f32, tag="ps_ne")
                for j in range(4):
                    nc.tensor.transpose(ps_ne[:, j, :],
                                        noiseT[:, j * 128:(j + 1) * 128],
                                        ident[0:64, 0:64])
                    nc.tensor.transpose(ps_ne[:, 4 + j, :],
                                        eoT[:, j * 128:(j + 1) * 128],
                                        ident[0:64, 0:64])
                for j in range(4):
                    v = ps_ne[:, j, :]
                    eo = ps_ne[:, 4 + j, :]
                    tv = small.tile([128, 8], f32, tag="tv")
                    nc.vector.max(tv, v)                      # top-8, descending
                    zap = small.tile([128, 64], f32, tag="zap")
                    nc.vector.match_replace(out=zap, in_to_replace=tv,
                                            in_values=v, imm_value=NEG_BIG)
                    negm1 = small.tile([128, 1], f32, tag="negm1")
                    nc.scalar.mul(negm1, tv[:, 0:1], -1.0)
                    # Z = sum of exp over the top-8 values
                    etv = small.tile([128, 8], f32, tag="etv")
                    zsum = small.tile([128, 1], f32, tag="zsum")
                    nc.scalar.activation(etv, tv, func=act.Exp,
                                         bias=negm1, accum_out=zsum)
                    e_all = small.tile([128, 64], f32, tag="e_all")
                    nc.scalar.activation(e_all, v, func=act.Exp, bias=negm1)
                    e_zap = small.tile([128, 64], f32, tag="e_zap")
                    nc.scalar.activation(e_zap, zap, func=act.Exp, bias=negm1)
                    # g = exp(v-m1) on top-8 positions, exactly 0 elsewhere
                    g = small.tile([128, 64], f32, tag="g")
                    nc.vector.tensor_sub(g, e_all, e_zap)
                    scr = small.tile([128, 64], f32, tag="scr")
                    nc.vector.tensor_mul(scr, g, eo)
                    ssum = small.tile([128, 1], f32, tag="ssum")
                    nc.vector.reduce_sum(ssum, scr, axis=mybir.AxisListType.X)
                    rz = small.tile([128, 1], f32, tag="rz")
                    nc.vector.reciprocal(rz, zsum)
                    nc.vector.tensor_mul(final_sb[:, t * 4 + j:t * 4 + j + 1],
                                         ssum, rz)

            # ---- output: [128, 16] -> [16, 128] -> DRAM [2048, 1] ----
            fin_ps = psfin.tile([16, 128], f32, tag="fin")
            nc.tensor.transpose(fin_ps, final_sb, ident)
            fin_t = eppool.tile([16, 128], f32, tag="fint")
            nc.scalar.copy(fin_t, fin_ps)
            nc.sync.dma_start(out=out.rearrange("(c p) o -> c (p o)", p=128),
                              in_=fin_t)

    nc.compile()
    return nc


def get_program(mm_dtype="fp16x2"):
    if mm_dtype not in _cached:
        _cached[mm_dtype] = _build_program(mm_dtype)
    return _cached[mm_dtype]


def make_in_maps(x, noise_w, noise_b, expert_w, expert_b, mm_dtype="fp16x2"):
    """Host-side sharding: per-core transposed x slice + replicated weights."""
    w_comb = np.concatenate([noise_w, expert_w], axis=0).astype(np.float32)  # [128, D]
    wt32 = np.ascontiguousarray(w_comb.T)                                    # [D, 128]
    bb = np.concatenate([noise_b, expert_b]).astype(np.float32).reshape(128, 1)
    if mm_dtype == "fp16x2":
        wh = wt32.astype(np.float16)
        wl = (wt32 - wh.astype(np.float32)).astype(np.float16)
        wp = np.stack([wh, wl], axis=1)                   # [D, 2, 128]
        # SBUF image: partition p holds [nk, 2, 128] for rows nk*128+p
        wt = np.ascontiguousarray(
            wp.reshape(NK, 128, 2, 128).transpose(1, 0, 2, 3).reshape(128, -1))
    else:
        wt = np.ascontiguousarray(
            wt32.reshape(NK, 128, 128).transpose(1, 0, 2).reshape(128, -1))
    in_maps = []
    for c in range(NCORES):
        xs = np.ascontiguousarray(x[c * BC:(c + 1) * BC, :].T)               # [D, BC]
        if mm_dtype == "fp16x2":
            xh = xs.astype(np.float16)
            xl = (xs - xh.astype(np.float32)).astype(np.float16)
            xs = np.ascontiguousarray(
                np.stack([xh.reshape(D, NT, BT), xl.reshape(D, NT, BT)],
                         axis=2))                                            # [D,NT,2,BT]
        else:
            xs = np.ascontiguousarray(xs.reshape(D, NT, BT))
        in_maps.append({"xt": xs, "wt": wt, "bb": bb})
    return in_maps


def kernel(x, noise, router_w, router_b, noise_w, noise_b, expert_w, expert_b,
           _trace=False):
    from concourse.bass_utils import run_bass_kernel_spmd

    x = np.asarray(x, dtype=np.float32)
    nc = get_program()
    in_maps = make_in_maps(x, np.asarray(noise_w), np.asarray(noise_b),
                           np.asarray(expert_w), np.asarray(expert_b))
    res = run_bass_kernel_spmd(nc, in_maps, core_ids=list(range(NCORES)),
                               trace=_trace)
    out = np.concatenate([r["out"] for r in res.results], axis=0)
    if _trace:
        kernel.last_results = res
    return out


# revision 30
# speedup vs baseline: 1.0407x; 1.0407x over previous
"""MoE logistic regression kernel for 8 Trainium2 NeuronCores.

Math (after dead-code elimination of the reference's unused router path):
    noise_logits = x @ noise_w.T + noise_b            # [B, E]
    top8 = top_k(noise_logits, 8)
    gates = softmax over the top-8 entries (others 0)
    expert = sigmoid(x @ expert_w.T + expert_b)       # [B, E]
    out[b] = sum_e gates[b,e] * expert[b,e]           # [B, 1]

Sharding: batch split 8 ways (2048 rows/core); weights replicated.

Key implementation choices:
- x is transposed on the host so each core streams contiguous [D, BC]
  chunks with D on partitions; no on-chip transpose of x.
- x and w are split into fp16 (hi, lo) pairs on the host (exact to ~22
  mantissa bits). The matmul runs 3 fp16 passes (hi@wh + lo@wh + hi@wl)
  accumulating in fp32 PSUM: ~fp32 accuracy at 3/4 the fp32 PE cost.
  (The top-8 selection margins require ~1e-6 logit accuracy: the
  smallest 8th/9th gap over the whole fixed batch is 8.8e-6.)
- noise_w/expert_w are concatenated into one 128-wide stationary operand
  so x streams through the PE once per (chunk, pass) for both matmuls;
  biases are added per-partition by the ACT epilogue ops.
- top-8 per row via the DVE Max8 + MatchReplace8 instructions; gates via
  exp(v - m1) with the (e_all - e_zap) trick which is exactly zero off
  the top-8; final dot + 1/Z normalization per 128-row tile.
"""

import sys

import numpy as np

if "/opt/trn_rl_repo" not in sys.path:
    sys.path.insert(0, "/opt/trn_rl_repo")

B, D, E, TOPK, NCORES = 16384, 4096, 64, 8, 8
BC = B // NCORES      # batch rows per core
BT = 512              # batch tile (one PSUM bank of fp32)
NT = BC // BT         # batch tiles per core
NK = D // 128         # contraction chunks
NEG_BIG = -1e30

_cached = {}


def _build_program(mm_dtype="fp16x2"):
    import concourse.bass as bass
    import concourse.tile as tile
    from concourse import bacc, mybir
    from concourse.masks import make_identity

    f32 = mybir.dt.float32
    f16 = mybir.dt.float16
    split = mm_dtype == "fp16x2"
    wdt = f16 if split else getattr(mybir.dt, mm_dtype)
    act = mybir.ActivationFunctionType

    nc = bacc.Bacc("TRN2", target_bir_lowering=False, debug=False)
    if split:
        # x as fp16 (hi, lo): [D, NT, 2, BT]; w pair pre-swizzled so the
        # SBUF image [128, NK*2*128] is one contiguous DMA.
        xt = nc.dram_tensor("xt", [D, NT, 2, BT], f16, kind="ExternalInput").ap()
        wt = nc.dram_tensor("wt", [128, NK * 2 * 128], f16,
                            kind="ExternalInput").ap()
    else:
        xt = nc.dram_tensor("xt", [D, NT, BT], f32, kind="ExternalInput").ap()
        wt = nc.dram_tensor("wt", [128, NK * 128], f32, kind="ExternalInput").ap()
    bb = nc.dram_tensor("bb", [128, 1], f32, kind="ExternalInput").ap()
    out = nc.dram_tensor("out", [BC, 1], f32, kind="ExternalOutput").ap()

    with tile.TileContext(nc) as tc:
        with (
            tc.tile_pool(name="consts", bufs=1) as consts,
            tc.tile_pool(name="xpool", bufs=5) as xpool,
            tc.tile_pool(name="eppool", bufs=2) as eppool,
            tc.tile_pool(name="small", bufs=3) as small,
            tc.tile_pool(name="psacc", bufs=1, space=bass.MemorySpace.PSUM) as psacc,
            tc.tile_pool(name="pstr", bufs=2, space=bass.MemorySpace.PSUM) as pstr,
            tc.tile_pool(name="psfin", bufs=1, space=bass.MemorySpace.PSUM) as psfin,
        ):
            # ---- constants ----
            if split:
                wt_sb = consts.tile([128, NK, 2, 128], wdt)
            else:
                wt_sb = consts.tile([128, NK, 128], wdt)
            nc.scalar.dma_start(out=wt_sb, in_=wt)
            bb_sb = consts.tile([128, 1], f32)
            nc.scalar.dma_start(out=bb_sb, in_=bb)
            ident = consts.tile([128, 128], f32)
            make_identity(nc, ident)
            final_sb = consts.tile([128, NT * 4], f32)

            # ---- matmuls: acc[t][0:64,:] = noise logits.T (pre-bias),
            #               acc[t][64:128,:] = expert logits.T (pre-bias)
            accs = [psacc.tile([128, BT], f32, tag=f"acc{t}", name=f"acc{t}")
                    for t in range(NT)]
            if split:
                # pair k-chunks: one 2MB DMA covers chunks 2kk and 2kk+1
                xview = xt.rearrange("(nkk two p) nt t b -> nkk p two nt t b",
                                     p=128, two=2)
                for kk in range(NK // 2):
                    dma_eng = nc.sync if kk % 2 == 0 else nc.scalar
                    xk = xpool.tile([128, 2, NT, 2, BT], wdt, tag="xk")
                    dma_eng.dma_start(out=xk, in_=xview[kk])
                    for c in range(2):
                        k = 2 * kk + c
                        wh = wt_sb[:, k, 0, :]
                        wl = wt_sb[:, k, 1, :]
                        for t in range(NT):
                            nc.tensor.matmul(accs[t], lhsT=wh,
                                             rhs=xk[:, c, t, 0, :],
                                             start=(k == 0), stop=False)
                            nc.tensor.matmul(accs[t], lhsT=wh,
                                             rhs=xk[:, c, t, 1, :],
                                             start=False, stop=False)
                            nc.tensor.matmul(accs[t], lhsT=wl,
                                             rhs=xk[:, c, t, 0, :],
                                             start=False,
                                             stop=(k == NK - 1))
            else:
                xview = xt.rearrange("(nk p) nt b -> nk p nt b", p=128)
                for k in range(NK):
                    xk = xpool.tile([128, NT, BT], wdt, tag="xk")
                    nc.sync.dma_start(out=xk, in_=xview[k])
                    for t in range(NT):
                        nc.tensor.matmul(accs[t], lhsT=wt_sb[:, k, :],
                                         rhs=xk[:, t, :],
                                         start=(k == 0), stop=(k == NK - 1))

            # ---- epilogue per batch tile ----
            for t in range(NT):
                noiseT = eppool.tile([64, BT], f32, tag="noiseT")
                nc.scalar.add(noiseT, accs[t][0:64, :], bb_sb[0:64, :])
                eoT = eppool.tile([64, BT], f32, tag="eoT")
                nc.scalar.activation(eoT, accs[t][64:128, :],
                                     func=act.Sigmoid, bias=bb_sb[64:128, :])
                # transpose to batch-major: [128 batch, j | 4+j, 64]
                ps_ne = pstr.tile([128, 8, 64], f32, tag="ps_ne")
                for j in range(4):
                    nc.tensor.transpose(ps_ne[:, j, :],
                                        noiseT[:, j * 128:(j + 1) * 128],
                                        ident[0:64, 0:64])
                    nc.tensor.transpose(ps_ne[:, 4 + j, :],
                                        eoT[:, j * 128:(j + 1) * 128],
                                        ident[0:64, 0:64])
                e_all = small.tile([128, 4, 64], f32, tag="e_all")
                e_zap = small.tile([128, 4, 64], f32, tag="e_zap")
                zsum = small.tile([128, 4], f32, tag="zsum")
                for j in range(4):
                    v = ps_ne[:, j, :]
                    tv = small.tile([128, 8], f32, tag="tv")
                    nc.vector.max(tv, v)                      # top-8, descending
                    zap = small.tile([128, 64], f32, tag="zap")
                    nc.vector.match_replace(out=zap, in_to_replace=tv,
                                            in_values=v, imm_value=NEG_BIG)
                    negm1 = small.tile([128, 1], f32, tag="negm1")
                    nc.scalar.mul(negm1, tv[:, 0:1], -1.0)
                    nc.scalar.activation(e_all[:, j, :], v, func=act.Exp,
                                         bias=negm1)
                    nc.scalar.activation(e_zap[:, j, :], zap, func=act.Exp,
                                         bias=negm1)
                # g = exp(v-m1) on top-8 positions, exactly 0 elsewhere;
                # grouped DVE math over all four 128-row subtiles at once
                g = small.tile([128, 4, 64], f32, tag="g")
                nc.vector.tensor_sub(g, e_all, e_zap)
                nc.vector.reduce_sum(zsum, g, axis=mybir.AxisListType.X)
                scr = small.tile([128, 4, 64], f32, tag="scr")
                nc.vector.tensor_mul(scr, g, ps_ne[:, 4:8, :])
                s4 = small.tile([128, 4], f32, tag="s4")
                nc.vector.reduce_sum(s4, scr, axis=mybir.AxisListType.X)
                rz = small.tile([128, 4], f32, tag="rz")
                nc.vector.reciprocal(rz, zsum)
                nc.vector.tensor_mul(final_sb[:, t * 4:(t + 1) * 4], s4, rz)

            # ---- output: [128, 16] -> [16, 128] -> DRAM [2048, 1] ----
            fin_ps = psfin.tile([16, 128], f32, tag="fin")
            nc.tensor.transpose(fin_ps, final_sb, ident)
            fin_t = eppool.tile([16, 128], f32, tag="fint")
            nc.scalar.copy(fin_t, fin_ps)
            nc.sync.dma_start(out=out.rearrange("(c p) o -> c (p o)", p=128),
                              in_=fin_t)

    nc.compile()
    return nc


def get_program(mm_dtype="fp16x2"):
    if mm_dtype not in _cached:
        _cached[mm_dtype] = _build_program(mm_dtype)
    return _cached[mm_dtype]


def make_in_maps(x, noise_w, noise_b, expert_w, expert_b, mm_dtype="fp16x2"):
    """Host-side sharding: per-core transposed x slice + replicated weights."""
    w_comb = np.concatenate([noise_w, expert_w], axis=0).astype(np.float32)  # [128, D]
    wt32 = np.ascontiguousarray(w_comb.T)                                    # [D, 128]
    bb = np.concatenate([noise_b, expert_b]).astype(np.float32).reshape(128, 1)
    if mm_dtype == "fp16x2":
        wh = wt32.astype(np.float16)
        wl = (wt32 - wh.astype(np.float32)).astype(np.float16)
        wp = np.stack([wh, wl], axis=1)                   # [D, 2, 128]
        # SBUF image: partition p holds [nk, 2, 128] for rows nk*128+p
        wt = np.ascontiguousarray(
            wp.reshape(NK, 128, 2, 128).transpose(1, 0, 2, 3).reshape(128, -1))
    else:
        wt = np.ascontiguousarray(
            wt32.reshape(NK, 128, 128).transpose(1, 0, 2).reshape(128, -1))
    in_maps = []
    for c in range(NCORES):
        xs = np.ascontiguousarray(x[c * BC:(c + 1) * BC, :].T)               # [D, BC]
        if mm_dtype == "fp16x2":
            xh = xs.astype(np.float16)
            xl = (xs - xh.astype(np.float32)).astype(np.float16)
            xs = np.ascontiguousarray(
                np.stack([xh.reshape(D, NT, BT), xl.reshape(D, NT, BT)],
                         axis=2))                                            # [D,NT,2,BT]
        else:
            xs = np.ascontiguousarray(xs.reshape(D, NT, BT))
        in_maps.append({"xt": xs, "wt": wt, "bb": bb})
    return in_maps


def kernel(x, noise, router_w, router_b, noise_w, noise_b, expert_w, expert_b,
           _trace=False):
    from concourse.bass_utils import run_bass_kernel_spmd

    x = np.asarray(x, dtype=np.float32)
    nc = get_program()
    in_maps = make_in_maps(x, np.asarray(noise_w), np.asarray(noise_b),
                           np.asarray(expert_w), np.asarray(expert_b))
    res = run_bass_kernel_spmd(nc, in_maps, core_ids=list(range(NCORES)),
                               trace=_trace)
    out = np.concatenate([r["out"] for r in res.results], axis=0)
    if _trace:
        kernel.last_results = res
    return out


# revision 33
# speedup vs baseline: 1.0543x; 1.0131x over previous
"""MoE logistic regression kernel for 8 Trainium2 NeuronCores.

Math (after dead-code elimination of the reference's unused router path):
    noise_logits = x @ noise_w.T + noise_b            # [B, E]
    top8 = top_k(noise_logits, 8)
    gates = softmax over the top-8 entries (others 0)
    expert = sigmoid(x @ expert_w.T + expert_b)       # [B, E]
    out[b] = sum_e gates[b,e] * expert[b,e]           # [B, 1]

Sharding: batch split 8 ways (2048 rows/core); weights replicated.

Key implementation choices:
- x is transposed on the host so each core streams contiguous [D, BC]
  chunks with D on partitions; no on-chip transpose of x.
- x and w are split into fp16 (hi, lo) pairs on the host (exact to ~22
  mantissa bits). The matmul runs 3 fp16 passes (hi@wh + lo@wh + hi@wl)
  accumulating in fp32 PSUM: ~fp32 accuracy at 3/4 the fp32 PE cost.
  (The top-8 selection margins require ~1e-6 logit accuracy: the
  smallest 8th/9th gap over the whole fixed batch is 8.8e-6.)
- noise_w/expert_w are concatenated into one 128-wide stationary operand
  so x streams through the PE once per (chunk, pass) for both matmuls;
  biases are added per-partition by the ACT epilogue ops.
- top-8 per row via the DVE Max8 + MatchReplace8 instructions; gates via
  exp(v - m1) with the (e_all - e_zap) trick which is exactly zero off
  the top-8; final dot + 1/Z normalization per 128-row tile.
"""

import sys

import numpy as np

if "/opt/trn_rl_repo" not in sys.path:
    sys.path.insert(0, "/opt/trn_rl_repo")

B, D, E, TOPK, NCORES = 16384, 4096, 64, 8, 8
BC = B // NCORES      # batch rows per core
BT = 512              # batch tile (one PSUM bank of fp32)
NT = BC // BT         # batch tiles per core
NK = D // 128         # contraction chunks
NEG_BIG = -1e30

_cached = {}


def _build_program(mm_dtype="fp16x2"):
    import concourse.bass as bass
    import concourse.tile as tile
    from concourse import bacc, mybir
    from concourse.masks import make_identity

    f32 = mybir.dt.float32
    f16 = mybir.dt.float16
    split = mm_dtype == "fp16x2"
    wdt = f16 if split else getattr(mybir.dt, mm_dtype)
    act = mybir.ActivationFunctionType

    nc = bacc.Bacc("TRN2", target_bir_lowering=False, debug=False)
    if split:
        # x as fp16 (hi, lo): [D, NT, 2, BT]; w pair pre-swizzled so the
        # SBUF image [128, NK*2*128] is one contiguous DMA.
        xt = nc.dram_tensor("xt", [D, NT, 2, BT], f16, kind="ExternalInput").ap()
        wt = nc.dram_tensor("wt", [128, NK * 2 * 128], f16,
                            kind="ExternalInput").ap()
    else:
        xt = nc.dram_tensor("xt", [D, NT, BT], f32, kind="ExternalInput").ap()
        wt = nc.dram_tensor("wt", [128, NK * 128], f32, kind="ExternalInput").ap()
    bb = nc.dram_tensor("bb", [128, 1], f32, kind="ExternalInput").ap()
    out = nc.dram_tensor("out", [BC, 1], f32, kind="ExternalOutput").ap()

    with tile.TileContext(nc) as tc:
        with (
            tc.tile_pool(name="consts", bufs=1) as consts,
            tc.tile_pool(name="xpool", bufs=5) as xpool,
            tc.tile_pool(name="eppool", bufs=2) as eppool,
            tc.tile_pool(name="small", bufs=3) as small,
            tc.tile_pool(name="psacc", bufs=1, space=bass.MemorySpace.PSUM) as psacc,
            tc.tile_pool(name="pstr", bufs=2, space=bass.MemorySpace.PSUM) as pstr,
            tc.tile_pool(name="psfin", bufs=1, space=bass.MemorySpace.PSUM) as psfin,
        ):
            # ---- constants ----
            if split:
                wt_sb = consts.tile([128, NK, 2, 128], wdt)
            else:
                wt_sb = consts.tile([128, NK, 128], wdt)
            nc.scalar.dma_start(out=wt_sb, in_=wt)
            bb_sb = consts.tile([128, 1], f32)
            nc.scalar.dma_start(out=bb_sb, in_=bb)
            ident = consts.tile([128, 128], f32)
            make_identity(nc, ident)
            final_sb = consts.tile([128, NT * 4], f32)

            # ---- matmuls: acc[t][0:64,:] = noise logits.T (pre-bias),
            #               acc[t][64:128,:] = expert logits.T (pre-bias)
            accs = [psacc.tile([128, BT], f32, tag=f"acc{t}", name=f"acc{t}")
                    for t in range(NT)]
            if split:
                # pair k-chunks: one 2MB DMA covers chunks 2kk and 2kk+1
                xview = xt.rearrange("(nkk two p) nt t b -> nkk p two nt t b",
                                     p=128, two=2)
                for kk in range(NK // 2):
                    dma_eng = nc.sync if kk % 2 == 0 else nc.scalar
                    xk = xpool.tile([128, 2, NT, 2, BT], wdt, tag="xk")
                    dma_eng.dma_start(out=xk, in_=xview[kk])
                    for c in range(2):
                        k = 2 * kk + c
                        wh = wt_sb[:, k, 0, :]
                        wl = wt_sb[:, k, 1, :]
                        for t in range(NT):
                            nc.tensor.matmul(accs[t], lhsT=wh,
                                             rhs=xk[:, c, t, 0, :],
                                             start=(k == 0), stop=False)
                            nc.tensor.matmul(accs[t], lhsT=wh,
                                             rhs=xk[:, c, t, 1, :],
                                             start=False, stop=False)
                            nc.tensor.matmul(accs[t], lhsT=wl,
                                             rhs=xk[:, c, t, 0, :],
                                             start=False,
                                             stop=(k == NK - 1))
            else:
                xview = xt.rearrange("(nk p) nt b -> nk p nt b", p=128)
                for k in range(NK):
                    xk = xpool.tile([128, NT, BT], wdt, tag="xk")
                    nc.sync.dma_start(out=xk, in_=xview[k])
                    for t in range(NT):
                        nc.tensor.matmul(accs[t], lhsT=wt_sb[:, k, :],
                                         rhs=xk[:, t, :],
                                         start=(k == 0), stop=(k == NK - 1))

            # ---- epilogue: pass 1 emits all bias/sigmoid + transposes so
            # the ACT FIFO isn't blocked by tile t's exp stream when tile
            # t+1's head ops become ready; pass 2 does the per-tile math.
            ps_nes = []
            for t in range(NT):
                noiseT = eppool.tile([64, BT], f32, tag="noiseT")
                nc.scalar.add(noiseT, accs[t][0:64, :], bb_sb[0:64, :])
                eoT = eppool.tile([64, BT], f32, tag="eoT")
                nc.scalar.activation(eoT, accs[t][64:128, :],
                                     func=act.Sigmoid, bias=bb_sb[64:128, :])
                # transpose to batch-major: [128 batch, j | 4+j, 64]
                ps_ne = pstr.tile([128, 8, 64], f32, tag="ps_ne",
                                  name=f"ps_ne{t}")
                for j in range(4):
                    nc.tensor.transpose(ps_ne[:, j, :],
                                        noiseT[:, j * 128:(j + 1) * 128],
                                        ident[0:64, 0:64])
                    nc.tensor.transpose(ps_ne[:, 4 + j, :],
                                        eoT[:, j * 128:(j + 1) * 128],
                                        ident[0:64, 0:64])
                ps_nes.append(ps_ne)
            for t in range(NT):
                ps_ne = ps_nes[t]
                e_all = small.tile([128, 4, 64], f32, tag="e_all")
                e_zap = small.tile([128, 4, 64], f32, tag="e_zap")
                zsum = small.tile([128, 4], f32, tag="zsum")
                for j in range(4):
                    v = ps_ne[:, j, :]
                    tv = small.tile([128, 8], f32, tag="tv")
                    nc.vector.max(tv, v)                      # top-8, descending
                    zap = small.tile([128, 64], f32, tag="zap")
                    nc.vector.match_replace(out=zap, in_to_replace=tv,
                                            in_values=v, imm_value=NEG_BIG)
                    negm1 = small.tile([128, 1], f32, tag="negm1")
                    nc.scalar.mul(negm1, tv[:, 0:1], -1.0)
                    nc.scalar.activation(e_all[:, j, :], v, func=act.Exp,
                                         bias=negm1)
                    nc.scalar.activation(e_zap[:, j, :], zap, func=act.Exp,
                                         bias=negm1)
                # g = exp(v-m1) on top-8 positions, exactly 0 elsewhere;
                # grouped DVE math over all four 128-row subtiles at once
                g = small.tile([128, 4, 64], f32, tag="g")
                nc.vector.tensor_sub(g, e_all, e_zap)
                nc.vector.reduce_sum(zsum, g, axis=mybir.AxisListType.X)
                scr = small.tile([128, 4, 64], f32, tag="scr")
                nc.vector.tensor_mul(scr, g, ps_ne[:, 4:8, :])
                s4 = small.tile([128, 4], f32, tag="s4")
                nc.vector.reduce_sum(s4, scr, axis=mybir.AxisListType.X)
                rz = small.tile([128, 4], f32, tag="rz")
                nc.vector.reciprocal(rz, zsum)
                nc.vector.tensor_mul(final_sb[:, t * 4:(t + 1) * 4], s4, rz)

            # ---- output: [128, 16] -> [16, 128] -> DRAM [2048, 1] ----
            fin_ps = psfin.tile([16, 128], f32, tag="fin")
            nc.tensor.transpose(fin_ps, final_sb, ident)
            fin_t = eppool.tile([16, 128], f32, tag="fint")
            nc.scalar.copy(fin_t, fin_ps)
            nc.sync.dma_start(out=out.rearrange("(c p) o -> c (p o)", p=128),
                              in_=fin_t)

    nc.compile()
    return nc


def get_program(mm_dtype="fp16x2"):
    if mm_dtype not in _cached:
        _cached[mm_dtype] = _build_program(mm_dtype)
    return _cached[mm_dtype]


def make_in_maps(x, noise_w, noise_b, expert_w, expert_b, mm_dtype="fp16x2"):
    """Host-side sharding: per-core transposed x slice + replicated weights."""
    w_comb = np.concatenate([noise_w, expert_w], axis=0).astype(np.float32)  # [128, D]
    wt32 = np.ascontiguousarray(w_comb.T)                                    # [D, 128]
    bb = np.concatenate([noise_b, expert_b]).astype(np.float32).reshape(128, 1)
    if mm_dtype == "fp16x2":
        wh = wt32.astype(np.float16)
        wl = (wt32 - wh.astype(np.float32)).astype(np.float16)
        wp = np.stack([wh, wl], axis=1)                   # [D, 2, 128]
        # SBUF image: partition p holds [nk, 2, 128] for rows nk*128+p
        wt = np.ascontiguousarray(
            wp.reshape(NK, 128, 2, 128).transpose(1, 0, 2, 3).reshape(128, -1))
    else:
        wt = np.ascontiguousarray(
            wt32.reshape(NK, 128, 128).transpose(1, 0, 2).reshape(128, -1))
    in_maps = []
    for c in range(NCORES):
        xs = np.ascontiguousarray(x[c * BC:(c + 1) * BC, :].T)               # [D, BC]
        if mm_dtype == "fp16x2":
            xh = xs.astype(np.float16)
            xl = (xs - xh.astype(np.float32)).astype(np.float16)
            xs = np.ascontiguousarray(
                np.stack([xh.reshape(D, NT, BT), xl.reshape(D, NT, BT)],
                         axis=2))                                            # [D,NT,2,BT]
        else:
            xs = np.ascontiguousarray(xs.reshape(D, NT, BT))
        in_maps.append({"xt": xs, "wt": wt, "bb": bb})
    return in_maps


def kernel(x, noise, router_w, router_b, noise_w, noise_b, expert_w, expert_b,
           _trace=False):
    from concourse.bass_utils import run_bass_kernel_spmd

    x = np.asarray(x, dtype=np.float32)
    nc = get_program()
    in_maps = make_in_maps(x, np.asarray(noise_w), np.asarray(noise_b),
                           np.asarray(expert_w), np.asarray(expert_b))
    res = run_bass_kernel_spmd(nc, in_maps, core_ids=list(range(NCORES)),
                               trace=_trace)
    out = np.concatenate([r["out"] for r in res.results], axis=0)
    if _trace:
        kernel.last_results = res
    return out


# revision 34
# speedup vs baseline: 1.0551x; 1.0008x over previous
"""MoE logistic regression kernel for 8 Trainium2 NeuronCores.

Math (after dead-code elimination of the reference's unused router path):
    noise_logits = x @ noise_w.T + noise_b            # [B, E]
    top8 = top_k(noise_logits, 8)
    gates = softmax over the top-8 entries (others 0)
    expert = sigmoid(x @ expert_w.T + expert_b)       # [B, E]
    out[b] = sum_e gates[b,e] * expert[b,e]           # [B, 1]

Sharding: batch split 8 ways (2048 rows/core); weights replicated.

Key implementation choices:
- x is transposed on the host so each core streams contiguous [D, BC]
  chunks with D on partitions; no on-chip transpose of x.
- x and w are split into fp16 (hi, lo) pairs on the host (exact to ~22
  mantissa bits). The matmul runs 3 fp16 passes (hi@wh + lo@wh + hi@wl)
  accumulating in fp32 PSUM: ~fp32 accuracy at 3/4 the fp32 PE cost.
  (The top-8 selection margins require ~1e-6 logit accuracy: the
  smallest 8th/9th gap over the whole fixed batch is 8.8e-6.)
- noise_w/expert_w are concatenated into one 128-wide stationary operand
  so x streams through the PE once per (chunk, pass) for both matmuls;
  biases are added per-partition by the ACT epilogue ops.
- top-8 per row via the DVE Max8 + MatchReplace8 instructions; gates via
  exp(v - m1) with the (e_all - e_zap) trick which is exactly zero off
  the top-8; final dot + 1/Z normalization per 128-row tile.
"""

import sys

import numpy as np

if "/opt/trn_rl_repo" not in sys.path:
    sys.path.insert(0, "/opt/trn_rl_repo")

B, D, E, TOPK, NCORES = 16384, 4096, 64, 8, 8
BC = B // NCORES      # batch rows per core
BT = 512              # batch tile (one PSUM bank of fp32)
NT = BC // BT         # batch tiles per core
NK = D // 128         # contraction chunks
NEG_BIG = -1e30

_cached = {}


def _build_program(mm_dtype="fp16x2"):
    import concourse.bass as bass
    import concourse.tile as tile
    from concourse import bacc, mybir
    from concourse.masks import make_identity

    f32 = mybir.dt.float32
    f16 = mybir.dt.float16
    split = mm_dtype == "fp16x2"
    wdt = f16 if split else getattr(mybir.dt, mm_dtype)
    act = mybir.ActivationFunctionType

    nc = bacc.Bacc("TRN2", target_bir_lowering=False, debug=False)
    if split:
        # x as fp16 (hi, lo): [D, NT, 2, BT]; w pair pre-swizzled so the
        # SBUF image [128, NK*2*128] is one contiguous DMA.
        xt = nc.dram_tensor("xt", [D, NT, 2, BT], f16, kind="ExternalInput").ap()
        wt = nc.dram_tensor("wt", [128, NK * 2 * 128], f16,
                            kind="ExternalInput").ap()
    else:
        xt = nc.dram_tensor("xt", [D, NT, BT], f32, kind="ExternalInput").ap()
        wt = nc.dram_tensor("wt", [128, NK * 128], f32, kind="ExternalInput").ap()
    bb = nc.dram_tensor("bb", [128, 1], f32, kind="ExternalInput").ap()
    out = nc.dram_tensor("out", [BC, 1], f32, kind="ExternalOutput").ap()

    with tile.TileContext(nc) as tc:
        with (
            tc.tile_pool(name="consts", bufs=1) as consts,
            tc.tile_pool(name="xpool", bufs=6) as xpool,
            tc.tile_pool(name="eppool", bufs=4) as eppool,
            tc.tile_pool(name="small", bufs=3) as small,
            tc.tile_pool(name="psacc", bufs=1, space=bass.MemorySpace.PSUM) as psacc,
            tc.tile_pool(name="pstr", bufs=2, space=bass.MemorySpace.PSUM) as pstr,
            tc.tile_pool(name="psfin", bufs=1, space=bass.MemorySpace.PSUM) as psfin,
        ):
            # ---- constants ----
            if split:
                wt_sb = consts.tile([128, NK, 2, 128], wdt)
            else:
                wt_sb = consts.tile([128, NK, 128], wdt)
            nc.scalar.dma_start(out=wt_sb, in_=wt)
            bb_sb = consts.tile([128, 1], f32)
            nc.scalar.dma_start(out=bb_sb, in_=bb)
            ident = consts.tile([128, 128], f32)
            make_identity(nc, ident)
            final_sb = consts.tile([128, NT * 4], f32)

            # ---- matmuls: acc[t][0:64,:] = noise logits.T (pre-bias),
            #               acc[t][64:128,:] = expert logits.T (pre-bias)
            accs = [psacc.tile([128, BT], f32, tag=f"acc{t}", name=f"acc{t}")
                    for t in range(NT)]
            if split:
                # pair k-chunks: one 2MB DMA covers chunks 2kk and 2kk+1
                xview = xt.rearrange("(nkk two p) nt t b -> nkk p two nt t b",
                                     p=128, two=2)
                for kk in range(NK // 2):
                    dma_eng = nc.sync if kk % 2 == 0 else nc.scalar
                    xk = xpool.tile([128, 2, NT, 2, BT], wdt, tag="xk")
                    dma_eng.dma_start(out=xk, in_=xview[kk])
                    for c in range(2):
                        k = 2 * kk + c
                        wh = wt_sb[:, k, 0, :]
                        wl = wt_sb[:, k, 1, :]
                        for t in range(NT):
                            nc.tensor.matmul(accs[t], lhsT=wh,
                                             rhs=xk[:, c, t, 0, :],
                                             start=(k == 0), stop=False)
                            nc.tensor.matmul(accs[t], lhsT=wh,
                                             rhs=xk[:, c, t, 1, :],
                                             start=False, stop=False)
                            nc.tensor.matmul(accs[t], lhsT=wl,
                                             rhs=xk[:, c, t, 0, :],
                                             start=False,
                                             stop=(k == NK - 1))
            else:
                xview = xt.rearrange("(nk p) nt b -> nk p nt b", p=128)
                for k in range(NK):
                    xk = xpool.tile([128, NT, BT], wdt, tag="xk")
                    nc.sync.dma_start(out=xk, in_=xview[k])
                    for t in range(NT):
                        nc.tensor.matmul(accs[t], lhsT=wt_sb[:, k, :],
                                         rhs=xk[:, t, :],
                                         start=(k == 0), stop=(k == NK - 1))

            # ---- epilogue: pass 1 emits all bias/sigmoid + transposes so
            # the ACT FIFO isn't blocked by tile t's exp stream when tile
            # t+1's head ops become ready; pass 2 does the per-tile math.
            ps_nes = []
            for t in range(NT):
                noiseT = eppool.tile([64, BT], f32, tag="noiseT")
                nc.scalar.add(noiseT, accs[t][0:64, :], bb_sb[0:64, :])
                eoT = eppool.tile([64, BT], f32, tag="eoT")
                nc.scalar.activation(eoT, accs[t][64:128, :],
                                     func=act.Sigmoid, bias=bb_sb[64:128, :])
                # transpose to batch-major: [128 batch, j | 4+j, 64]
                ps_ne = pstr.tile([128, 8, 64], f32, tag="ps_ne",
                                  name=f"ps_ne{t}")
                for j in range(4):
                    nc.tensor.transpose(ps_ne[:, j, :],
                                        noiseT[:, j * 128:(j + 1) * 128],
                                        ident[0:64, 0:64])
                    nc.tensor.transpose(ps_ne[:, 4 + j, :],
                                        eoT[:, j * 128:(j + 1) * 128],
                                        ident[0:64, 0:64])
                ps_nes.append(ps_ne)
            for t in range(NT):
                ps_ne = ps_nes[t]
                e_all = small.tile([128, 4, 64], f32, tag="e_all")
                e_zap = small.tile([128, 4, 64], f32, tag="e_zap")
                zsum = small.tile([128, 4], f32, tag="zsum")
                for j in range(4):
                    v = ps_ne[:, j, :]
                    tv = small.tile([128, 8], f32, tag="tv")
                    nc.vector.max(tv, v)                      # top-8, descending
                    zap = small.tile([128, 64], f32, tag="zap")
                    nc.vector.match_replace(out=zap, in_to_replace=tv,
                                            in_values=v, imm_value=NEG_BIG)
                    negm1 = small.tile([128, 1], f32, tag="negm1")
                    nc.scalar.mul(negm1, tv[:, 0:1], -1.0)
                    nc.scalar.activation(e_all[:, j, :], v, func=act.Exp,
                                         bias=negm1)
                    nc.scalar.activation(e_zap[:, j, :], zap, func=act.Exp,
                                         bias=negm1)
                # g = exp(v-m1) on top-8 positions, exactly 0 elsewhere;
                # grouped DVE math over all four 128-row subtiles at once
                g = small.tile([128, 4, 64], f32, tag="g")
                nc.vector.tensor_sub(g, e_all, e_zap)
                nc.vector.reduce_sum(zsum, g, axis=mybir.AxisListType.X)
                scr = small.tile([128, 4, 64], f32, tag="scr")
                nc.vector.tensor_mul(scr, g, ps_ne[:, 4:8, :])
                s4 = small.tile([128, 4], f32, tag="s4")
                nc.vector.reduce_sum(s4, scr, axis=mybir.AxisListType.X)
                rz = small.tile([128, 4], f32, tag="rz")
                nc.vector.reciprocal(rz, zsum)
                nc.vector.tensor_mul(final_sb[:, t * 4:(t + 1) * 4], s4, rz)

            # ---- output: [128, 16] -> [16, 128] -> DRAM [2048, 1] ----
            fin_ps = psfin.tile([16, 128], f32, tag="fin")
            nc.tensor.transpose(fin_ps, final_sb, ident)
            fin_t = eppool.tile([16, 128], f32, tag="fint")
            nc.scalar.copy(fin_t, fin_ps)
            nc.sync.dma_start(out=out.rearrange("(c p) o -> c (p o)", p=128),
                              in_=fin_t)

    nc.compile()
    return nc


def get_program(mm_dtype="fp16x2"):
    if mm_dtype not in _cached:
        _cached[mm_dtype] = _build_program(mm_dtype)
    return _cached[mm_dtype]


def make_in_maps(x, noise_w, noise_b, expert_w, expert_b, mm_dtype="fp16x2"):
    """Host-side sharding: per-core transposed x slice + replicated weights."""
    w_comb = np.concatenate([noise_w, expert_w], axis=0).astype(np.float32)  # [128, D]
    wt32 = np.ascontiguousarray(w_comb.T)                                    # [D, 128]
    bb = np.concatenate([noise_b, expert_b]).astype(np.float32).reshape(128, 1)
    if mm_dtype == "fp16x2":
        wh = wt32.astype(np.float16)
        wl = (wt32 - wh.astype(np.float32)).astype(np.float16)
        wp = np.stack([wh, wl], axis=1)                   # [D, 2, 128]
        # SBUF image: partition p holds [nk, 2, 128] for rows nk*128+p
        wt = np.ascontiguousarray(
            wp.reshape(NK, 128, 2, 128).transpose(1, 0, 2, 3).reshape(128, -1))
    else:
        wt = np.ascontiguousarray(
            wt32.reshape(NK, 128, 128).transpose(1, 0, 2).reshape(128, -1))
    in_maps = []
    for c in range(NCORES):
        xs = np.ascontiguousarray(x[c * BC:(c + 1) * BC, :].T)               # [D, BC]
        if mm_dtype == "fp16x2":
            xh = xs.astype(np.float16)
            xl = (xs - xh.astype(np.float32)).astype(np.float16)
            xs = np.ascontiguousarray(
                np.stack([xh.reshape(D, NT, BT), xl.reshape(D, NT, BT)],
                         axis=2))                                            # [D,NT,2,BT]
        else:
            xs = np.ascontiguousarray(xs.reshape(D, NT, BT))
        in_maps.append({"xt": xs, "wt": wt, "bb": bb})
    return in_maps


def kernel(x, noise, router_w, router_b, noise_w, noise_b, expert_w, expert_b,
           _trace=False):
    from concourse.bass_utils import run_bass_kernel_spmd

    x = np.asarray(x, dtype=np.float32)
    nc = get_program()
    in_maps = make_in_maps(x, np.asarray(noise_w), np.asarray(noise_b),
                           np.asarray(expert_w), np.asarray(expert_b))
    res = run_bass_kernel_spmd(nc, in_maps, core_ids=list(range(NCORES)),
                               trace=_trace)
    out = np.concatenate([r["out"] for r in res.results], axis=0)
    if _trace:
        kernel.last_results = res
    return out
